# revision 10
# baseline (speedup 1.0000x reference)
"""Trainium2 Bass kernel for nn_MultiHeadAttention_40870908789096.

MHA with a 2D log-bucketed relative-position bias, key masking, softmax.

Strategy (8 cores; core c handles batch b=c//2, head-group hg=c%2 i.e. 4 heads):
  - scores kept TRANSPOSED: [k partitions, q free]. k-axis sorted by x-coord,
    q-axis sorted by y-coord (host permutations; undone on output).
  - The RPE bias fx_h(bucket(x_q-x_k)) + fy_h(bucket(y_q-y_k)) is piecewise
    constant with breakpoints precomputed on host (searchsorted per threshold):
      * x-bias: piecewise along the (x-sorted) k axis -> built as a sparse
        delta plane via gpsimd local_scatter, then cumsum'd ALONG PARTITIONS
        by a triangular-ones matmul accumulated directly into the score PSUM.
      * y-bias: piecewise along the (y-sorted) q axis -> sparse delta plane
        via local_scatter, cumsum along free dim via DVE tensor_tensor_scan,
        added into score PSUM via an identity matmul.
  - No softmax max-pass (scores are O(5), exp is safe in f32); no sum-reduce:
    V is augmented with a ones-column (pre-multiplied by the key mask, which
    also zeroes masked V rows - exactly equivalent to the -inf score mask),
    so the PV matmul yields both the context and the softmax denominator.
  - exp via ACT with scale=1/8 (bias tables pre-scaled by 8).
  - Out-projection partial per core; host sums core pairs, un-permutes, + bo.
"""

import math
from contextlib import ExitStack

import ml_dtypes
import numpy as np

import concourse.bass as bass
import concourse.mybir as mybir
from concourse import bacc
import concourse.tile as tile
from concourse.bass_utils import run_bass_kernel_spmd
from concourse.masks import make_identity, make_upper_triangular

BF16 = mybir.dt.bfloat16
F32 = mybir.dt.float32
I16 = mybir.dt.int16
NPBF16 = ml_dtypes.bfloat16

B, G, D, H, DH = 4, 1024, 512, 8, 64
HPC = 4  # heads per core
NCORES = 8
NKT = 8  # k tiles of 128
QHW = 512  # q half width
NSX = 192  # x-scatter slots per head per row
NSY = 16  # y-scatter slots per head per row
NUM_BUCKETS = 32

# ---------------------------------------------------------------- host math


def _log_index_np(n):
    ln = np.log(n.astype(np.float32)).astype(np.float32)
    q = (ln / np.float32(math.log(2.0))).astype(np.float32)
    return np.clip(np.floor(q), 0, NUM_BUCKETS - 1).astype(np.int32)


def _bucket_np(delta):
    """Reference bucket in [24, 38]; returns local index s in [0, 14]."""
    delta = np.asarray(delta, np.float32)
    s = np.sign(delta).astype(np.int32)
    n = np.clip(np.abs(delta), np.float32(1e-6), np.float32(128.0)).astype(np.float32)
    return _log_index_np(n) * s + 7


_CUTS = None


def _compute_cuts():
    """14 indicator thresholds t=1..14 for s(delta) >= t.

    Returns list of (bval: f64, incl_tie: bool). Indicator_t(d) for f32-rounded
    d = x_q - x_k is: (d_real > bval) or (d_real == bval and incl_tie).
    """
    global _CUTS
    if _CUTS is not None:
        return _CUTS
    mags = []
    for j in range(1, 8):
        lo = np.float32(2.0**j)
        for _ in range(300):
            lo = np.float32(np.nextafter(lo, np.float32(0)))
        cand = [lo]
        for _ in range(600):
            cand.append(np.float32(np.nextafter(cand[-1], np.float32(np.inf))))
        cand = np.array(cand, np.float32)
        n = np.clip(np.abs(cand), np.float32(1e-6), np.float32(128.0)).astype(np.float32)
        li = _log_index_np(n)
        mags.append(np.float32(cand[li >= j].min()))
    cuts = []
    for t in range(1, 15):
        if t <= 7:
            c = mags[8 - t - 1]
            pred = np.float32(np.nextafter(c, np.float32(0)))
            w = (float(c) + float(pred)) / 2.0
            even = (c.view(np.uint32) & np.uint32(1)) == 0
            cuts.append((-w, not bool(even)))
        else:
            c = mags[t - 7 - 1]
            pred = np.float32(np.nextafter(c, np.float32(0)))
            w = (float(c) + float(pred)) / 2.0
            even = (c.view(np.uint32) & np.uint32(1)) == 0
            cuts.append((w, bool(even)))
    _CUTS = cuts
    return cuts


def _geometry(coords_b):
    """Per-batch sort perms and boundary rank arrays.

    Returns dict with:
      pq, pk          : permutations (q by y, k by x)
      A [14, G] int   : x-axis boundary ranks along k (per q column)
      a [14, G] int   : y-axis boundary ranks along q (per k row)
    """
    cuts = _compute_cuts()
    coords_b = np.asarray(coords_b, np.float32)
    pq = np.argsort(coords_b[:, 1], kind="stable")
    pk = np.argsort(coords_b[:, 0], kind="stable")
    xq = coords_b[pq, 0].astype(np.float64)
    yqs = coords_b[pq, 1].astype(np.float64)
    xks = coords_b[pk, 0].astype(np.float64)
    yk = coords_b[pk, 1].astype(np.float64)
    A = np.empty((14, G), np.int64)
    a = np.empty((14, G), np.int64)
    for t0, (bval, incl) in enumerate(cuts):
        A[t0] = np.searchsorted(xks, xq - bval, side="right" if incl else "left")
        a[t0] = np.searchsorted(yqs, yk + bval, side="left" if incl else "right")
    return {"pq": pq, "pk": pk, "A": A, "a": a}


def _x_scatter(A, FXh):
    """Build x-side scatter arrays + vxinit for 4 heads.

    A: [14, G] boundary k-ranks per q column. FXh: [4, 15] level tables.
    Returns dxi [NKT,2,2,128,2*NSX] i16, dxd same bf16, vxinit [4*NKT, G] bf16.
    """
    dxi = np.full((NKT, 2, 2, 128, 2 * NSX), -1, np.int16)
    dxd = np.zeros((NKT, 2, 2, 128, 2 * NSX), np.float32)

    t_idx = np.repeat(np.arange(14), G)
    q_idx = np.tile(np.arange(G), 14)
    g = A.reshape(-1)
    valid = (g >= 1) & (g <= G - 1) & (g % 128 != 0)
    t_v, q_v, g_v = t_idx[valid], q_idx[valid], g[valid]
    # delta value for event t0 (indicator t0+1 turns OFF as k ascends):
    # s goes t0+1 -> t0 ==> dval[h] = FXh[h, t0] - FXh[h, t0+1]
    dvals = FXh[:, t_v] - FXh[:, t_v + 1]  # [4, nev]
    # merge duplicates at same (g, q)
    key = g_v * G + q_v
    order = np.argsort(key, kind="stable")
    key_s = key[order]
    uniq = np.ones(len(key_s), bool)
    uniq[1:] = key_s[1:] != key_s[:-1]
    seg = np.cumsum(uniq) - 1
    nseg = int(seg[-1]) + 1 if len(seg) else 0
    mval = np.zeros((4, nseg), np.float32)
    for h in range(4):
        mval[h] = np.bincount(seg, weights=dvals[h][order], minlength=nseg)
    g_s = g_v[order][uniq]
    q_s = q_v[order][uniq]
    mval = mval[:, :]  # [4, nseg] aligned with uniq seg order
    # slot index within (tile, row, qh)
    tile_i = g_s // 128
    row_i = g_s % 128
    qh_i = q_s // QHW
    qcol = q_s % QHW
    grp = (tile_i * 128 + row_i) * 2 + qh_i
    o2 = np.argsort(grp, kind="stable")
    grp_s = grp[o2]
    first = np.ones(len(grp_s), bool)
    first[1:] = grp_s[1:] != grp_s[:-1]
    start_of_grp = np.maximum.accumulate(np.where(first, np.arange(len(grp_s)), 0))
    slot = np.arange(len(grp_s)) - start_of_grp
    if len(slot) and slot.max() >= NSX:
        raise RuntimeError(f"NSX overflow: need {slot.max() + 1}")
    ti, ri, qi, qc = tile_i[o2], row_i[o2], qh_i[o2], qcol[o2]
    for pr in range(2):
        for sh in range(2):
            h = 2 * pr + sh
            dxi[ti, qi, pr, ri, NSX * sh + slot] = (QHW * sh + qc).astype(np.int16)
            dxd[ti, qi, pr, ri, NSX * sh + slot] = mval[h][o2]
    # vxinit[h, m, q] = FXh[h, s_at(m, q)], s_at = #{t: A_t > 128m}
    s_at = (A[None, :, :] > (128 * np.arange(NKT))[:, None, None]).sum(1)  # [NKT, G]
    vxinit = np.empty((4, NKT, G), np.float32)
    for h in range(4):
        vxinit[h] = FXh[h][s_at]
    return (
        dxi,
        dxd.astype(NPBF16),
        vxinit.reshape(1, 4 * NKT * G).astype(NPBF16),
    )


def _y_scatter(a, FYh):
    """Build y-side scatter arrays for 4 heads.

    a: [14, G] boundary q-ranks per k row. FYh: [4, 15].
    Returns dyi [NKT,2,2,128,2*NSY] i16, dyd bf16.
    """
    dyi = np.full((NKT, 2, 2, 128, 2 * NSY), -1, np.int16)
    dyd = np.zeros((NKT, 2, 2, 128, 2 * NSY), np.float32)
    rows = np.arange(G)
    for qh in range(2):
        awin = a - QHW * qh  # [14, G]
        s_start = (a <= QHW * qh).sum(0)  # [G]
        s_end = (a <= QHW * qh + QHW - 1).sum(0)
        for pr in range(2):
            for sh in range(2):
                h = 2 * pr + sh
                idx_w = np.full((G, NSY), -1, np.int64)
                val_w = np.zeros((G, NSY), np.float32)
                # init/seam slot at window position 0 of this head segment
                idx_w[:, 0] = QHW * sh
                if sh == 0:
                    val_w[:, 0] = FYh[h][s_start]
                else:
                    val_w[:, 0] = FYh[h][s_start] - FYh[h - 1][s_end]
                last_pos = np.zeros(G, np.int64)
                last_slot = np.zeros(G, np.int64)
                for t0 in range(14):
                    p = awin[t0]
                    inw = (p >= 1) & (p <= QHW - 1)
                    dv = FYh[h, t0 + 1] - FYh[h, t0]
                    same = inw & (p == last_pos)
                    new = inw & ~same
                    val_w[rows[same], last_slot[same]] += dv
                    ls = last_slot.copy()
                    ls[new] += 1
                    idx_w[rows[new], ls[new]] = QHW * sh + p[new]
                    val_w[rows[new], ls[new]] = dv
                    last_slot = ls
                    last_pos = np.where(new, p, last_pos)
                if last_slot.max() >= NSY:
                    raise RuntimeError(f"NSY overflow: need {last_slot.max() + 1}")
                dyi[:, qh, pr, :, NSY * sh : NSY * (sh + 1)] = idx_w.reshape(
                    NKT, 128, NSY
                ).astype(np.int16)
                dyd[:, qh, pr, :, NSY * sh : NSY * (sh + 1)] = val_w.reshape(NKT, 128, NSY)
    return dyi, dyd.astype(NPBF16)


def host_prep(inputs):
    """Build per-core in_maps + per-batch q-permutations."""
    query = np.asarray(inputs["query"], np.float32)
    key = np.asarray(inputs["key"], np.float32)
    value = np.asarray(inputs["value"], np.float32)
    coords = np.asarray(inputs["coords"], np.float32)
    mask = np.asarray(inputs["mask"])
    Wq, bq = np.asarray(inputs["Wq"], np.float32), np.asarray(inputs["bq"], np.float32)
    Wk, bk = np.asarray(inputs["Wk"], np.float32), np.asarray(inputs["bk"], np.float32)
    Wv, bv = np.asarray(inputs["Wv"], np.float32), np.asarray(inputs["bv"], np.float32)
    Wo = np.asarray(inputs["Wo"], np.float32)
    rpe_x = np.asarray(inputs["rpe_x"], np.float32)
    rpe_y = np.asarray(inputs["rpe_y"], np.float32)

    FX = 8.0 * rpe_x[24:39, :].T.astype(np.float32)  # [H, 15]
    FY = 8.0 * rpe_y[24:39, :].T.astype(np.float32)

    geoms = [_geometry(coords[b]) for b in range(B)]
    in_maps = []
    perms = []
    for b in range(B):
        gm = geoms[b]
        pq, pk = gm["pq"], gm["pk"]
        perms.append(pq)
        qT = np.ascontiguousarray(query[b][pq].T).astype(NPBF16)  # [D, G]
        kT = np.ascontiguousarray(key[b][pk].T).astype(NPBF16)
        vT = np.ascontiguousarray(value[b][pk].T).astype(NPBF16)
        maskf = mask[b][pk].astype(np.float32).reshape(G, 1)
        for hg in range(2):
            sl = slice(hg * 256, hg * 256 + 256)
            hsl = slice(hg * 4, hg * 4 + 4)
            dxi, dxd, vxinit = _x_scatter(gm["A"], FX[hsl])
            dyi, dyd = _y_scatter(gm["a"], FY[hsl])
            in_maps.append(
                {
                    "qT": qT,
                    "kT": kT,
                    "vT": vT,
                    "wqT": np.ascontiguousarray(Wq.T[:, sl]).astype(NPBF16),
                    "wkT": np.ascontiguousarray(Wk.T[:, sl]).astype(NPBF16),
                    "wvT": np.ascontiguousarray(Wv.T[:, sl]).astype(NPBF16),
                    "woT": np.ascontiguousarray(Wo.T[sl, :]).astype(NPBF16),
                    "bqs": bq[sl].reshape(256, 1).astype(np.float32),
                    "bks": bk[sl].reshape(256, 1).astype(np.float32),
                    "bvs": bv[sl].reshape(1, 256).astype(NPBF16),
                    "maskf": maskf,
                    "vxin": vxinit,
                    "dxi": dxi,
                    "dxd": dxd,
                    "dyi": dyi,
                    "dyd": dyd,
                }
            )
    return in_maps, perms, np.asarray(inputs["bo"], np.float32)


# ---------------------------------------------------------------- device


_NC_CACHE = {}


def build_nc():
    if "nc" in _NC_CACHE:
        return _NC_CACHE["nc"]
    nc = bacc.Bacc("TRN2", target_bir_lowering=False, debug=False, num_devices=1)

    def din(name, shape, dt):
        return nc.dram_tensor(name, shape, dt, kind="ExternalInput").ap()

    qTd = din("qT", [D, G], BF16)
    kTd = din("kT", [D, G], BF16)
    vTd = din("vT", [D, G], BF16)
    wqTd = din("wqT", [D, 256], BF16)
    wkTd = din("wkT", [D, 256], BF16)
    wvTd = din("wvT", [D, 256], BF16)
    woTd = din("woT", [256, D], BF16)
    bqsd = din("bqs", [256, 1], F32)
    bksd = din("bks", [256, 1], F32)
    bvsd = din("bvs", [1, 256], BF16)
    maskd = din("maskf", [G, 1], F32)
    vxind = din("vxin", [1, HPC * NKT * G], BF16)
    dxid = din("dxi", [NKT, 2, 2, 128, 2 * NSX], I16)
    dxdd = din("dxd", [NKT, 2, 2, 128, 2 * NSX], BF16)
    dyid = din("dyi", [NKT, 2, 2, 128, 2 * NSY], I16)
    dydd = din("dyd", [NKT, 2, 2, 128, 2 * NSY], BF16)
    outd = nc.dram_tensor("outp", [D, G], F32, kind="ExternalOutput").ap()

    EXP = mybir.ActivationFunctionType.Exp
    CPY = mybir.ActivationFunctionType.Copy
    ADD = mybir.AluOpType.add
    MUL = mybir.AluOpType.mult
    BYP = mybir.AluOpType.bypass

    with tile.TileContext(nc) as tc, ExitStack() as ctx:
        sb = ctx.enter_context(tc.tile_pool(name="persist", bufs=1))

        # --- persistent SBUF tensors
        xq = sb.tile([128, 4, G], BF16)
        xk = sb.tile([128, 4, G], BF16)
        xv = sb.tile([128, 4, G], BF16)
        wq = sb.tile([128, 4, 256], BF16)
        wk = sb.tile([128, 4, 256], BF16)
        wv = sb.tile([128, 4, 256], BF16)
        wo = sb.tile([128, 2, D], BF16)
        bqt = sb.tile([128, 2, 1], F32)
        bkt = sb.tile([128, 2, 1], F32)
        bvt = sb.tile([1, 256], BF16)
        maskt = sb.tile([128, NKT, 1], F32)
        vxin = sb.tile([1, HPC * NKT * G], BF16)
        QTs = sb.tile([128, 2, G], BF16)
        KTs = sb.tile([128, 2, G], BF16)
        vm = sb.tile([128, NKT, HPC, 65], BF16)
        ctxn = sb.tile([128, 2, G], BF16)
        ones1 = sb.tile([1, 128], BF16)
        ones1f = sb.tile([1, 128], F32)
        tril = sb.tile([128, 128], BF16)
        iden = sb.tile([128, 128], BF16)

        nc.vector.memset(ones1, 1.0)
        nc.vector.memset(ones1f, 1.0)
        make_upper_triangular(nc, tril, val=1.0, diag=True)
        make_identity(nc, iden)

        for c in range(4):
            nc.sync.dma_start(out=xq[:, c, :], in_=qTd[128 * c : 128 * (c + 1), :])
            nc.sync.dma_start(out=xk[:, c, :], in_=kTd[128 * c : 128 * (c + 1), :])
            nc.sync.dma_start(out=xv[:, c, :], in_=vTd[128 * c : 128 * (c + 1), :])
            nc.sync.dma_start(out=wq[:, c, :], in_=wqTd[128 * c : 128 * (c + 1), :])
            nc.sync.dma_start(out=wk[:, c, :], in_=wkTd[128 * c : 128 * (c + 1), :])
            nc.sync.dma_start(out=wv[:, c, :], in_=wvTd[128 * c : 128 * (c + 1), :])
        for c in range(2):
            nc.sync.dma_start(out=wo[:, c, :], in_=woTd[128 * c : 128 * (c + 1), :])
            nc.sync.dma_start(out=bqt[:, c, :], in_=bqsd[128 * c : 128 * (c + 1), :])
            nc.sync.dma_start(out=bkt[:, c, :], in_=bksd[128 * c : 128 * (c + 1), :])
        nc.sync.dma_start(out=bvt, in_=bvsd)
        nc.sync.dma_start(out=vxin, in_=vxind)
        for kt in range(NKT):
            nc.sync.dma_start(out=maskt[:, kt, :], in_=maskd[128 * kt : 128 * (kt + 1), :])

        # --- projections
        with tc.tile_pool(name="ppsum", bufs=2, space=bass.MemorySpace.PSUM) as pp:
            for w_in, b_in, dst in ((wq, bqt, QTs), (wk, bkt, KTs)):
                src = xq if dst is QTs else xk
                for mt in range(2):
                    for nh in range(2):
                        ps = pp.tile([128, 512], F32, tag="pj")
                        for c in range(4):
                            nc.tensor.matmul(
                                ps,
                                lhsT=w_in[:, c, 128 * mt : 128 * (mt + 1)],
                                rhs=src[:, c, QHW * nh : QHW * (nh + 1)],
                                start=(c == 0),
                                stop=(c == 3),
                            )
                        nc.vector.tensor_scalar(
                            out=dst[:, mt, QHW * nh : QHW * (nh + 1)],
                            in0=ps,
                            scalar1=b_in[:, mt, :],
                            scalar2=None,
                            op0=ADD,
                        )
            for kt in range(NKT):
                ps2 = pp.tile([128, 256], F32, tag="pv")
                for c in range(4):
                    nc.tensor.matmul(
                        ps2,
                        lhsT=xv[:, c, 128 * kt : 128 * (kt + 1)],
                        rhs=wv[:, c, :],
                        start=(c == 0),
                        stop=False,
                    )
                nc.tensor.matmul(ps2, lhsT=ones1, rhs=bvt, start=False, stop=True)
                for hl in range(HPC):
                    nc.vector.tensor_scalar(
                        out=vm[:, kt, hl, 0:64],
                        in0=ps2[:, 64 * hl : 64 * (hl + 1)],
                        scalar1=maskt[:, kt, :],
                        scalar2=None,
                        op0=MUL,
                    )
                    nc.vector.tensor_copy(vm[:, kt, hl, 64:65], maskt[:, kt, :])

        # --- attention
        for qh in range(2):
            with (
                tc.tile_pool(name="pctx", bufs=4, space=bass.MemorySpace.PSUM) as pctx,
                tc.tile_pool(name="psc", bufs=2, space=bass.MemorySpace.PSUM) as psc,
                tc.tile_pool(name="pw", bufs=3) as pw,
            ):
                ctxp = [pctx.tile([65, 512], F32, tag="ctx", name=f"ctx{qh}_{i}") for i in range(HPC)]
                for kt in range(NKT):
                    xi = pw.tile([128, 2, 2 * NSX], I16, tag="xi")
                    xd = pw.tile([128, 2, 2 * NSX], BF16, tag="xd")
                    yi = pw.tile([128, 2, 2 * NSY], I16, tag="yi")
                    yd = pw.tile([128, 2, 2 * NSY], BF16, tag="yd")
                    for pr in range(2):
                        nc.sync.dma_start(out=xi[:, pr, :], in_=dxid[kt, qh, pr])
                        nc.sync.dma_start(out=xd[:, pr, :], in_=dxdd[kt, qh, pr])
                        nc.sync.dma_start(out=yi[:, pr, :], in_=dyid[kt, qh, pr])
                        nc.sync.dma_start(out=yd[:, pr, :], in_=dydd[kt, qh, pr])
                    dx = [pw.tile([128, 1024], BF16, tag=f"dx{pr}", name=f"dx_{qh}_{kt}_{pr}") for pr in range(2)]
                    dy = [pw.tile([128, 1024], BF16, tag=f"dy{pr}", name=f"dy_{qh}_{kt}_{pr}") for pr in range(2)]
                    vy = [pw.tile([128, 1024], BF16, tag=f"vy{pr}", name=f"vy_{qh}_{kt}_{pr}") for pr in range(2)]
                    for pr in range(2):
                        nc.gpsimd.local_scatter(
                            dx[pr], xd[:, pr, :], xi[:, pr, :],
                            channels=128, num_elems=1024, num_idxs=2 * NSX,
                        )
                        nc.gpsimd.local_scatter(
                            dy[pr], yd[:, pr, :], yi[:, pr, :],
                            channels=128, num_elems=1024, num_idxs=2 * NSY,
                        )
                        nc.vector.tensor_tensor_scan(
                            out=vy[pr], data0=dy[pr], data1=dy[pr],
                            initial=0.0, op0=ADD, op1=BYP,
                        )
                    for hl in range(HPC):
                        pr, sh = hl // 2, hl % 2
                        sc = psc.tile([128, 512], F32, tag="sc")
                        nc.tensor.matmul(
                            sc,
                            lhsT=KTs[64 * sh : 64 * (sh + 1), pr, 128 * kt : 128 * (kt + 1)],
                            rhs=QTs[64 * sh : 64 * (sh + 1), pr, QHW * qh : QHW * (qh + 1)],
                            start=True,
                            stop=False,
                        )
                        nc.tensor.matmul(
                            sc, lhsT=tril, rhs=dx[pr][:, QHW * sh : QHW * (sh + 1)],
                            start=False, stop=False,
                        )
                        nc.tensor.matmul(
                            sc,
                            lhsT=ones1,
                            rhs=vxin[0:1, (hl * NKT + kt) * G + QHW * qh : (hl * NKT + kt) * G + QHW * (qh + 1)],
                            start=False,
                            stop=False,
                        )
                        nc.tensor.matmul(
                            sc, lhsT=iden, rhs=vy[pr][:, QHW * sh : QHW * (sh + 1)],
                            start=False, stop=True,
                        )
                        eh = pw.tile([128, 512], BF16, tag="eh")
                        nc.scalar.activation(eh, sc, EXP, scale=0.125)
                        nc.tensor.matmul(
                            ctxp[hl], lhsT=vm[:, kt, hl, :], rhs=eh,
                            start=(kt == 0), stop=(kt == NKT - 1),
                        )
                # normalize: ctxn[:, :, qh half] = ctx / rowsum
                for hl in range(HPC):
                    pr, sh = hl // 2, hl % 2
                    rec = pw.tile([1, 512], F32, tag="rec")
                    nc.vector.reciprocal(rec, ctxp[hl][64:65, :])
                    bcp = psc.tile([64, 512], F32, tag="bcp")
                    nc.tensor.matmul(bcp, lhsT=ones1f[:, 0:64], rhs=rec, start=True, stop=True)
                    bc = pw.tile([64, 512], F32, tag="bc")
                    nc.scalar.activation(bc, bcp, CPY)
                    nc.vector.tensor_mul(
                        ctxn[64 * sh : 64 * (sh + 1), pr, QHW * qh : QHW * (qh + 1)],
                        ctxp[hl][0:64, :],
                        bc,
                    )
            # out-projection for this q half
            with (
                tc.tile_pool(name="po", bufs=2, space=bass.MemorySpace.PSUM) as po,
                tc.tile_pool(name="pob", bufs=2) as pob,
            ):
                for mt in range(4):
                    pso = po.tile([128, 512], F32, tag="po")
                    for c2 in range(2):
                        nc.tensor.matmul(
                            pso,
                            lhsT=wo[:, c2, 128 * mt : 128 * (mt + 1)],
                            rhs=ctxn[:, c2, QHW * qh : QHW * (qh + 1)],
                            start=(c2 == 0),
                            stop=(c2 == 1),
                        )
                    ob = pob.tile([128, 512], F32, tag="ob")
                    nc.scalar.activation(ob, pso, CPY)
                    nc.sync.dma_start(
                        out=outd[128 * mt : 128 * (mt + 1), QHW * qh : QHW * (qh + 1)], in_=ob
                    )

    nc.compile()
    _NC_CACHE["nc"] = nc
    return nc


# ---------------------------------------------------------------- entry


def kernel(**inputs):
    in_maps, perms, bo = host_prep(inputs)
    nc = build_nc()
    res = run_bass_kernel_spmd(nc, in_maps, core_ids=list(range(NCORES)))
    out = np.zeros((B, G, D), np.float32)
    for b in range(B):
        acc = res.results[2 * b]["outp"] + res.results[2 * b + 1]["outp"]  # [D, G]
        out[b, perms[b], :] = acc.T
    out += bo[None, None, :]
    return out


# revision 11
# speedup vs baseline: 1.3510x; 1.3510x over previous
"""Trainium2 Bass kernel for nn_MultiHeadAttention_40870908789096.

MHA with a 2D log-bucketed relative-position bias, key masking, softmax.

Strategy (8 cores; core c handles batch b=c//2, head-group hg=c%2 i.e. 4 heads):
  - scores kept TRANSPOSED: [k partitions, q free]. k-axis sorted by x-coord,
    q-axis sorted by y-coord (host permutations; undone on output).
  - The RPE bias fx_h(bucket(x_q-x_k)) + fy_h(bucket(y_q-y_k)) is piecewise
    constant along each sorted axis; the host precomputes sparse boundary
    DELTA planes (breakpoints via searchsorted per threshold, f32-exact):
      * x-bias: delta plane along the (x-sorted) k axis -> cumsum'd ALONG
        PARTITIONS by a triangular-ones matmul accumulated directly into the
        score PSUM (row 0 of each 128-tile carries the absolute init row).
      * y-bias: delta plane along the (y-sorted) q axis -> cumsum along the
        free dim on DVE (tensor_tensor_scan), added into the score PSUM via
        an identity matmul.
  - No softmax max-pass (scores are O(5), exp is safe in f32); no sum-reduce:
    V is augmented with a ones-column (pre-multiplied by the key mask, which
    also zeroes masked V rows - exactly equivalent to the -inf score mask),
    so the PV matmul yields both the context and the softmax denominator.
  - exp via ACT with scale=1/8 (bias tables pre-scaled by 8).
  - Out-projection partial per core; host sums core pairs, un-permutes, + bo.
"""

import math
from contextlib import ExitStack

import ml_dtypes
import numpy as np

import concourse.bass as bass
import concourse.mybir as mybir
from concourse import bacc
import concourse.tile as tile
from concourse.bass_utils import run_bass_kernel_spmd
from concourse.masks import make_identity, make_upper_triangular

BF16 = mybir.dt.bfloat16
F32 = mybir.dt.float32
NPBF16 = ml_dtypes.bfloat16

B, G, D, H, DH = 4, 1024, 512, 8, 64
HPC = 4  # heads per core
NCORES = 8
NKT = 8  # k tiles of 128
QHW = 512  # q half width
NUM_BUCKETS = 32

# ---------------------------------------------------------------- host math


def _log_index_np(n):
    ln = np.log(n.astype(np.float32)).astype(np.float32)
    q = (ln / np.float32(math.log(2.0))).astype(np.float32)
    return np.clip(np.floor(q), 0, NUM_BUCKETS - 1).astype(np.int32)


def _bucket_np(delta):
    """Reference bucket - 24, i.e. local index s in [0, 14]."""
    delta = np.asarray(delta, np.float32)
    s = np.sign(delta).astype(np.int32)
    n = np.clip(np.abs(delta), np.float32(1e-6), np.float32(128.0)).astype(np.float32)
    return _log_index_np(n) * s + 7


_CUTS = None


def _compute_cuts():
    """14 indicator thresholds t=1..14 for s(delta) >= t.

    Returns list of (bval: f64, incl_tie: bool). Indicator_t(d) for f32-rounded
    d = x_q - x_k is: (d_real > bval) or (d_real == bval and incl_tie).
    """
    global _CUTS
    if _CUTS is not None:
        return _CUTS
    mags = []
    for j in range(1, 8):
        lo = np.float32(2.0**j)
        for _ in range(300):
            lo = np.float32(np.nextafter(lo, np.float32(0)))
        cand = [lo]
        for _ in range(600):
            cand.append(np.float32(np.nextafter(cand[-1], np.float32(np.inf))))
        cand = np.array(cand, np.float32)
        n = np.clip(np.abs(cand), np.float32(1e-6), np.float32(128.0)).astype(np.float32)
        li = _log_index_np(n)
        mags.append(np.float32(cand[li >= j].min()))
    cuts = []
    for t in range(1, 15):
        if t <= 7:
            c = mags[8 - t - 1]
            pred = np.float32(np.nextafter(c, np.float32(0)))
            w = (float(c) + float(pred)) / 2.0
            even = (c.view(np.uint32) & np.uint32(1)) == 0
            cuts.append((-w, not bool(even)))
        else:
            c = mags[t - 7 - 1]
            pred = np.float32(np.nextafter(c, np.float32(0)))
            w = (float(c) + float(pred)) / 2.0
            even = (c.view(np.uint32) & np.uint32(1)) == 0
            cuts.append((w, bool(even)))
    _CUTS = cuts
    return cuts


def _geometry(coords_b):
    """Per-batch sort perms and boundary rank arrays.

    Returns dict with:
      pq, pk          : permutations (q by y, k by x)
      A [14, G] int   : x-axis boundary ranks along k (per q column)
      a [14, G] int   : y-axis boundary ranks along q (per k row)
    """
    cuts = _compute_cuts()
    coords_b = np.asarray(coords_b, np.float32)
    pq = np.argsort(coords_b[:, 1], kind="stable")
    pk = np.argsort(coords_b[:, 0], kind="stable")
    xq = coords_b[pq, 0].astype(np.float64)
    yqs = coords_b[pq, 1].astype(np.float64)
    xks = coords_b[pk, 0].astype(np.float64)
    yk = coords_b[pk, 1].astype(np.float64)
    A = np.empty((14, G), np.int64)
    a = np.empty((14, G), np.int64)
    for t0, (bval, incl) in enumerate(cuts):
        A[t0] = np.searchsorted(xks, xq - bval, side="right" if incl else "left")
        a[t0] = np.searchsorted(yqs, yk + bval, side="left" if incl else "right")
    return {"pq": pq, "pk": pk, "A": A, "a": a}


def _x_planes(A, FXh):
    """Dense x-delta planes for 4 heads: [NKT, 2qh, 2pr, 128, 1024] bf16.

    Layout: plane[kt, qh, pr, row, 512*sh + qcol], head h = 2*pr + sh.
    Row 0 of each k-tile holds the absolute bias at that tile's first k row
    (the triangular-ones matmul adds row 0 into every row's prefix sum).
    Rows 1..127 hold fx deltas at within-tile boundary rows.
    """
    dxp = np.zeros((NKT, 2, 2, 128, 1024), np.float32)
    # events: indicator t0+1 turns OFF at k-rank g => value FX[t0] - FX[t0+1]
    t_idx = np.repeat(np.arange(14), G)
    q_idx = np.tile(np.arange(G), 14)
    g = A.reshape(-1)
    valid = (g >= 1) & (g <= G - 1) & (g % 128 != 0)
    t_v, q_v, g_v = t_idx[valid], q_idx[valid], g[valid]
    ti = g_v // 128
    ri = g_v % 128
    qi = q_v // QHW
    qc = q_v % QHW
    for pr in range(2):
        for sh in range(2):
            h = 2 * pr + sh
            dval = FXh[h, t_v] - FXh[h, t_v + 1]
            np.add.at(dxp, (ti, qi, pr, ri, QHW * sh + qc), dval)
    # init rows: s_at[m, q] = #{t: A_t(q) > 128m}
    s_at = (A[None, :, :] > (128 * np.arange(NKT))[:, None, None]).sum(1)  # [NKT, G]
    for pr in range(2):
        for sh in range(2):
            h = 2 * pr + sh
            vals = FXh[h][s_at]  # [NKT, G]
            for qh in range(2):
                dxp[:, qh, pr, 0, QHW * sh : QHW * (sh + 1)] = vals[
                    :, QHW * qh : QHW * (qh + 1)
                ]
    return dxp.astype(NPBF16)


def _y_planes(a, FYh):
    """Dense y-delta planes for 4 heads: [NKT, 2qh, 2pr, 128, 1024] bf16.

    Per row (k) the plane holds fy deltas at q-boundary positions; position 0
    of segment sh=0 holds the absolute init value, position 0 of segment sh=1
    holds the seam correction (scan runs across the whole 1024-wide row).
    """
    dyp = np.zeros((NKT, 2, 2, 128, 1024), np.float32)
    k_idx = np.tile(np.arange(G), 14)
    t_idx = np.repeat(np.arange(14), G)
    av = a.reshape(-1)
    for qh in range(2):
        p = av - QHW * qh
        inw = (p >= 1) & (p <= QHW - 1)
        kv, tv, pv = k_idx[inw], t_idx[inw], p[inw]
        s_start = (a <= QHW * qh).sum(0)  # [G]
        s_end = (a <= QHW * qh + QHW - 1).sum(0)
        for pr in range(2):
            for sh in range(2):
                h = 2 * pr + sh
                dval = FYh[h, tv + 1] - FYh[h, tv]
                np.add.at(dyp, (kv // 128, qh, pr, kv % 128, QHW * sh + pv), dval)
                init = (
                    FYh[h][s_start]
                    if sh == 0
                    else FYh[h][s_start] - FYh[h - 1][s_end]
                )
                dyp[:, qh, pr, :, QHW * sh] += init.reshape(NKT, 128)
    return dyp.astype(NPBF16)


def host_prep(inputs):
    """Build per-core in_maps + per-batch q-permutations."""
    query = np.asarray(inputs["query"], np.float32)
    key = np.asarray(inputs["key"], np.float32)
    value = np.asarray(inputs["value"], np.float32)
    coords = np.asarray(inputs["coords"], np.float32)
    mask = np.asarray(inputs["mask"])
    Wq, bq = np.asarray(inputs["Wq"], np.float32), np.asarray(inputs["bq"], np.float32)
    Wk, bk = np.asarray(inputs["Wk"], np.float32), np.asarray(inputs["bk"], np.float32)
    Wv, bv = np.asarray(inputs["Wv"], np.float32), np.asarray(inputs["bv"], np.float32)
    Wo = np.asarray(inputs["Wo"], np.float32)
    rpe_x = np.asarray(inputs["rpe_x"], np.float32)
    rpe_y = np.asarray(inputs["rpe_y"], np.float32)

    FX = 8.0 * rpe_x[24:39, :].T.astype(np.float32)  # [H, 15]
    FY = 8.0 * rpe_y[24:39, :].T.astype(np.float32)

    in_maps = []
    perms = []
    for b in range(B):
        gm = _geometry(coords[b])
        pq, pk = gm["pq"], gm["pk"]
        perms.append(pq)
        qT = np.ascontiguousarray(query[b][pq].T).astype(NPBF16)  # [D, G]
        kT = np.ascontiguousarray(key[b][pk].T).astype(NPBF16)
        vT = np.ascontiguousarray(value[b][pk].T).astype(NPBF16)
        maskf = mask[b][pk].astype(np.float32).reshape(G, 1)
        for hg in range(2):
            sl = slice(hg * 256, hg * 256 + 256)
            hsl = slice(hg * 4, hg * 4 + 4)
            in_maps.append(
                {
                    "qT": qT,
                    "kT": kT,
                    "vT": vT,
                    "wqT": np.ascontiguousarray(Wq.T[:, sl]).astype(NPBF16),
                    "wkT": np.ascontiguousarray(Wk.T[:, sl]).astype(NPBF16),
                    "wvT": np.ascontiguousarray(Wv.T[:, sl]).astype(NPBF16),
                    "woT": np.ascontiguousarray(Wo.T[sl, :]).astype(NPBF16),
                    "bqs": bq[sl].reshape(256, 1).astype(np.float32),
                    "bks": bk[sl].reshape(256, 1).astype(np.float32),
                    "bvs": bv[sl].reshape(1, 256).astype(NPBF16),
                    "maskf": maskf,
                    "dxp": _x_planes(gm["A"], FX[hsl]),
                    "dyp": _y_planes(gm["a"], FY[hsl]),
                }
            )
    return in_maps, perms, np.asarray(inputs["bo"], np.float32)


# ---------------------------------------------------------------- device


_NC_CACHE = {}


def build_nc():
    if "nc" in _NC_CACHE:
        return _NC_CACHE["nc"]
    nc = bacc.Bacc("TRN2", target_bir_lowering=False, debug=False, num_devices=1)

    def din(name, shape, dt):
        return nc.dram_tensor(name, shape, dt, kind="ExternalInput").ap()

    qTd = din("qT", [D, G], BF16)
    kTd = din("kT", [D, G], BF16)
    vTd = din("vT", [D, G], BF16)
    wqTd = din("wqT", [D, 256], BF16)
    wkTd = din("wkT", [D, 256], BF16)
    wvTd = din("wvT", [D, 256], BF16)
    woTd = din("woT", [256, D], BF16)
    bqsd = din("bqs", [256, 1], F32)
    bksd = din("bks", [256, 1], F32)
    bvsd = din("bvs", [1, 256], BF16)
    maskd = din("maskf", [G, 1], F32)
    dxpd = din("dxp", [NKT, 2, 2, 128, 1024], BF16)
    dypd = din("dyp", [NKT, 2, 2, 128, 1024], BF16)
    outd = nc.dram_tensor("outp", [D, G], F32, kind="ExternalOutput").ap()

    EXP = mybir.ActivationFunctionType.Exp
    CPY = mybir.ActivationFunctionType.Copy
    ADD = mybir.AluOpType.add
    MUL = mybir.AluOpType.mult
    BYP = mybir.AluOpType.bypass

    with tile.TileContext(nc) as tc, ExitStack() as ctx:
        sb = ctx.enter_context(tc.tile_pool(name="persist", bufs=1))

        # --- persistent SBUF tensors
        xq = sb.tile([128, 4, G], BF16)
        xk = sb.tile([128, 4, G], BF16)
        xv = sb.tile([128, 4, G], BF16)
        wq = sb.tile([128, 4, 256], BF16)
        wk = sb.tile([128, 4, 256], BF16)
        wv = sb.tile([128, 4, 256], BF16)
        wo = sb.tile([128, 2, D], BF16)
        bqt = sb.tile([128, 2, 1], F32)
        bkt = sb.tile([128, 2, 1], F32)
        bvt = sb.tile([1, 256], BF16)
        maskt = sb.tile([128, NKT, 1], F32)
        QTs = sb.tile([128, 2, G], BF16)
        KTs = sb.tile([128, 2, G], BF16)
        vm = sb.tile([128, NKT, HPC, 65], BF16)
        ctxn = sb.tile([128, 2, G], BF16)
        ones1 = sb.tile([1, 128], BF16)
        ones1f = sb.tile([1, 128], F32)
        tril = sb.tile([128, 128], BF16)
        iden = sb.tile([128, 128], BF16)

        nc.vector.memset(ones1, 1.0)
        nc.vector.memset(ones1f, 1.0)
        make_upper_triangular(nc, tril, val=1.0, diag=True)
        make_identity(nc, iden)

        for c in range(4):
            nc.sync.dma_start(out=xq[:, c, :], in_=qTd[128 * c : 128 * (c + 1), :])
            nc.sync.dma_start(out=xk[:, c, :], in_=kTd[128 * c : 128 * (c + 1), :])
            nc.sync.dma_start(out=xv[:, c, :], in_=vTd[128 * c : 128 * (c + 1), :])
            nc.sync.dma_start(out=wq[:, c, :], in_=wqTd[128 * c : 128 * (c + 1), :])
            nc.sync.dma_start(out=wk[:, c, :], in_=wkTd[128 * c : 128 * (c + 1), :])
            nc.sync.dma_start(out=wv[:, c, :], in_=wvTd[128 * c : 128 * (c + 1), :])
        for c in range(2):
            nc.sync.dma_start(out=wo[:, c, :], in_=woTd[128 * c : 128 * (c + 1), :])
            nc.sync.dma_start(out=bqt[:, c, :], in_=bqsd[128 * c : 128 * (c + 1), :])
            nc.sync.dma_start(out=bkt[:, c, :], in_=bksd[128 * c : 128 * (c + 1), :])
        nc.sync.dma_start(out=bvt, in_=bvsd)
        for kt in range(NKT):
            nc.sync.dma_start(out=maskt[:, kt, :], in_=maskd[128 * kt : 128 * (kt + 1), :])

        # --- projections
        with tc.tile_pool(name="ppsum", bufs=2, space=bass.MemorySpace.PSUM) as pp:
            for w_in, b_in, dst in ((wq, bqt, QTs), (wk, bkt, KTs)):
                src = xq if dst is QTs else xk
                for mt in range(2):
                    for nh in range(2):
                        ps = pp.tile([128, 512], F32, tag="pj")
                        for c in range(4):
                            nc.tensor.matmul(
                                ps,
                                lhsT=w_in[:, c, 128 * mt : 128 * (mt + 1)],
                                rhs=src[:, c, QHW * nh : QHW * (nh + 1)],
                                start=(c == 0),
                                stop=(c == 3),
                            )
                        nc.vector.tensor_scalar(
                            out=dst[:, mt, QHW * nh : QHW * (nh + 1)],
                            in0=ps,
                            scalar1=b_in[:, mt, :],
                            scalar2=None,
                            op0=ADD,
                        )
            for kt in range(NKT):
                ps2 = pp.tile([128, 256], F32, tag="pv")
                for c in range(4):
                    nc.tensor.matmul(
                        ps2,
                        lhsT=xv[:, c, 128 * kt : 128 * (kt + 1)],
                        rhs=wv[:, c, :],
                        start=(c == 0),
                        stop=False,
                    )
                nc.tensor.matmul(ps2, lhsT=ones1, rhs=bvt, start=False, stop=True)
                for hl in range(HPC):
                    nc.vector.tensor_scalar(
                        out=vm[:, kt, hl, 0:64],
                        in0=ps2[:, 64 * hl : 64 * (hl + 1)],
                        scalar1=maskt[:, kt, :],
                        scalar2=None,
                        op0=MUL,
                    )
                    nc.vector.tensor_copy(vm[:, kt, hl, 64:65], maskt[:, kt, :])

        # --- attention
        for qh in range(2):
            with (
                tc.tile_pool(name="pctx", bufs=4, space=bass.MemorySpace.PSUM) as pctx,
                tc.tile_pool(name="psc", bufs=2, space=bass.MemorySpace.PSUM) as psc,
                tc.tile_pool(name="pw", bufs=3) as pw,
            ):
                ctxp = [
                    pctx.tile([65, 512], F32, tag="ctx", name=f"ctx{qh}_{i}")
                    for i in range(HPC)
                ]
                for kt in range(NKT):
                    dx = [
                        pw.tile([128, 1024], BF16, tag=f"dx{pr}", name=f"dx_{qh}_{kt}_{pr}")
                        for pr in range(2)
                    ]
                    dy = [
                        pw.tile([128, 1024], BF16, tag=f"dy{pr}", name=f"dy_{qh}_{kt}_{pr}")
                        for pr in range(2)
                    ]
                    vy = [
                        pw.tile([128, 1024], BF16, tag=f"vy{pr}", name=f"vy_{qh}_{kt}_{pr}")
                        for pr in range(2)
                    ]
                    for pr in range(2):
                        nc.sync.dma_start(out=dx[pr], in_=dxpd[kt, qh, pr])
                        nc.sync.dma_start(out=dy[pr], in_=dypd[kt, qh, pr])
                        nc.vector.tensor_tensor_scan(
                            out=vy[pr], data0=dy[pr], data1=dy[pr],
                            initial=0.0, op0=ADD, op1=BYP,
                        )
                    for hl in range(HPC):
                        pr, sh = hl // 2, hl % 2
                        sc = psc.tile([128, 512], F32, tag="sc")
                        nc.tensor.matmul(
                            sc,
                            lhsT=KTs[64 * sh : 64 * (sh + 1), pr, 128 * kt : 128 * (kt + 1)],
                            rhs=QTs[64 * sh : 64 * (sh + 1), pr, QHW * qh : QHW * (qh + 1)],
                            start=True,
                            stop=False,
                        )
                        nc.tensor.matmul(
                            sc, lhsT=tril, rhs=dx[pr][:, QHW * sh : QHW * (sh + 1)],
                            start=False, stop=False,
                        )
                        nc.tensor.matmul(
                            sc, lhsT=iden, rhs=vy[pr][:, QHW * sh : QHW * (sh + 1)],
                            start=False, stop=True,
                        )
                        eh = pw.tile([128, 512], BF16, tag="eh")
                        nc.scalar.activation(eh, sc, EXP, scale=0.125)
                        nc.tensor.matmul(
                            ctxp[hl], lhsT=vm[:, kt, hl, :], rhs=eh,
                            start=(kt == 0), stop=(kt == NKT - 1),
                        )
                # normalize: ctxn[:, :, qh half] = ctx / rowsum
                for hl in range(HPC):
                    pr, sh = hl // 2, hl % 2
                    rec = pw.tile([1, 512], F32, tag="rec")
                    nc.vector.reciprocal(rec, ctxp[hl][64:65, :])
                    bcp = psc.tile([64, 512], F32, tag="bcp")
                    nc.tensor.matmul(bcp, lhsT=ones1f[:, 0:64], rhs=rec, start=True, stop=True)
                    bc = pw.tile([64, 512], F32, tag="bc")
                    nc.scalar.activation(bc, bcp, CPY)
                    nc.vector.tensor_mul(
                        ctxn[64 * sh : 64 * (sh + 1), pr, QHW * qh : QHW * (qh + 1)],
                        ctxp[hl][0:64, :],
                        bc,
                    )
            # out-projection for this q half
            with (
                tc.tile_pool(name="po", bufs=2, space=bass.MemorySpace.PSUM) as po,
                tc.tile_pool(name="pob", bufs=2) as pob,
            ):
                for mt in range(4):
                    pso = po.tile([128, 512], F32, tag="po")
                    for c2 in range(2):
                        nc.tensor.matmul(
                            pso,
                            lhsT=wo[:, c2, 128 * mt : 128 * (mt + 1)],
                            rhs=ctxn[:, c2, QHW * qh : QHW * (qh + 1)],
                            start=(c2 == 0),
                            stop=(c2 == 1),
                        )
                    ob = pob.tile([128, 512], F32, tag="ob")
                    nc.scalar.activation(ob, pso, CPY)
                    nc.sync.dma_start(
                        out=outd[128 * mt : 128 * (mt + 1), QHW * qh : QHW * (qh + 1)], in_=ob
                    )

    nc.compile()
    _NC_CACHE["nc"] = nc
    return nc


# ---------------------------------------------------------------- entry


def kernel(**inputs):
    in_maps, perms, bo = host_prep(inputs)
    nc = build_nc()
    res = run_bass_kernel_spmd(nc, in_maps, core_ids=list(range(NCORES)))
    out = np.zeros((B, G, D), np.float32)
    for b in range(B):
        acc = res.results[2 * b]["outp"] + res.results[2 * b + 1]["outp"]  # [D, G]
        out[b, perms[b], :] = acc.T
    out += bo[None, None, :]
    return out
